# revision 26
# baseline (speedup 1.0000x reference)
"""Self-contained Trainium2 Bass kernel for nn_AttentionModel (B=4, S=2048, E=1024).

Model: q/k/v linear projections + scaled-dot-product attention (scale = sqrt(E)).

Sharding (8 NeuronCores): core c handles batch b=c//2, query-row half h=c%2
(1024 q rows). k/v projections are split across the core pair (each projects
its own 1024 k-rows) and exchanged with a pair-local AllGather
(replica_groups [[0,1],[2,3],[4,5],[6,7]]); both halves are read back from
the collective output in absolute rank order, so the program stays fully
SPMD-uniform with no core-dependent addressing.

Device algorithm per core (all matmuls bf16 with fp32 PSUM accumulation):
  qT_proj[f,q]  = WqT.T @ qT_in   (+bq via ACT bias on eviction)
  kT_proj[f,k]  = WkT.T @ kT_in   (+bk)   [own half, then pair AllGather]
  v_proj [k,f]  = vT_in.T @ WvT   [own half, then pair AllGather;
                                   bias bv applied on host after gather]
  scoresT[k,q]  = kT_proj.T @ qT_proj          (per 128k x 512q psum tile)
  expT   [k,q]  = exp(scoresT / sqrt(E))       (ACT, no max-subtraction:
                                                logits are O(+-6), fp32-safe)
  out_un [q,f]  = expT.T @ v_proj              (accumulate over k chunks)
  sums   [q]    = DVE-accumulated exp tiles, partition-folded by a tiny
                  fp32 ones-matmul into per-partition [q,1] layout
  out    [q,f]  = out_un * (1/sums)            (per-partition ACT scale)

Scheduling notes (from perfetto analysis of the previous version):
  - Input priming is split across BOTH HWDGE engines (sync + scalar) so
    descriptor generation and transfers overlap; the k-proj inputs (kt
    halves on scalar, wk fo0-1 first on sync) land first so the PE starts
    ~13us in instead of ~21us.
  - k chains evict into a CONTIGUOUS staging tile (not kt_proj), so each
    kb half is staged to the collective input with ONE contiguous DMA and
    the AllGathers launch much earlier.  kt_proj is written only by the
    collective readbacks.  v staging is one contiguous DMA as well.
  - Readbacks ride the sync queue after priming drains; everything is
    resident long before the consuming phase, which removes the v-readback
    stall + half-rate region that used to sit at the attn@V entry.
  - attn@V runs the pa (f 0:512) chain fully, evicts + stores that half,
    then the pb chain, halving the post-last-matmul tail.

Host pre-tiles every input into fully-contiguous-per-partition bf16 SBUF
layouts, so the device performs no transposes or casts on the inputs.
"""

import sys

for _p in ("/opt/trn_rl_repo", "/root/.axon_site/_ro/trn_rl_repo"):
    if _p not in sys.path:
        sys.path.insert(0, _p)

import numpy as np
import ml_dtypes

import concourse.bacc as bacc
import concourse.mybir as mybir
import concourse.tile as tile
from concourse.bass_utils import run_bass_kernel_spmd

B, S, E = 4, 2048, 1024
P = 128
SQ = S // 2          # q rows per core
N_CORES = 8
EO = E // P          # 8  e-outer chunks
FO = E // P          # 8  f-outer chunks
KC = S // P          # 16 k-row chunks
QB = SQ // 512       # 2  q 512-blocks
KB = SQ // 512       # 2  k 512-blocks (own half)
INV_SCALE = float(1.0 / np.sqrt(np.float32(E)))

BF16 = mybir.dt.bfloat16
F32 = mybir.dt.float32

_BUILD_CACHE: dict = {}


def _build(with_mask: bool):
    nc = bacc.Bacc(
        "TRN2",
        target_bir_lowering=False,
        debug=False,
        enable_asserts=False,
        num_devices=N_CORES,
    )

    # Host-pretiled inputs; every transfer below is contiguous per partition.
    qt_d = nc.declare_dram_parameter("qt", [P, QB, EO, 512], BF16, isOutput=False)
    kt_d = nc.declare_dram_parameter("kt", [P, KB, EO, 512], BF16, isOutput=False)
    vt_d = nc.declare_dram_parameter("vt", [P, KC // 2, EO, P], BF16, isOutput=False)
    wqt_d = nc.declare_dram_parameter("wqt", [P, FO, EO, P], BF16, isOutput=False)
    wkt_d = nc.declare_dram_parameter("wkt", [P, FO, EO, P], BF16, isOutput=False)
    wvt_d = nc.declare_dram_parameter("wvt", [P, EO, E], BF16, isOutput=False)
    bq_d = nc.declare_dram_parameter("bq", [P, FO], F32, isOutput=False)
    bk_d = nc.declare_dram_parameter("bk", [P, FO], F32, isOutput=False)
    if with_mask:
        mask_d = nc.declare_dram_parameter("maskt", [P, KC, SQ], BF16, isOutput=False)
    # bf16 output: the attention output is an attn-weighted average (values
    # ~N(0, 0.03)); bf16 rounding adds ~0.4% relative error on top of the
    # ~0.6% bf16-compute error — far inside the 2e-2 gate — and halves the
    # output DMA traffic + final-store landing time.
    out_d = nc.declare_dram_parameter("out", [P, SQ // P, E], BF16, isOutput=True)

    # pair-AllGather staging buffers (internal DRAM)
    # NOTE: Shared-output collectives need >4-core groups; for 2-core pair
    # groups the output must stay a Local internal tensor. The k exchange is
    # split into two kb-halves so the first AllGather launches early and the
    # CC stream stays busy while phase A continues.
    cck_in = [nc.dram_tensor(f"cck_in{kb}", [P, FO, 512], BF16) for kb in range(KB)]
    cck_out = [nc.dram_tensor(f"cck_out{kb}", [2, P, FO, 512], BF16)
               for kb in range(KB)]
    ccv_in = [nc.dram_tensor(f"ccv_in{h}", [P, KC // 4, E], BF16) for h in range(2)]
    ccv_out = [nc.dram_tensor(f"ccv_out{h}", [2, P, KC // 4, E], BF16)
               for h in range(2)]
    REPLICA_GROUPS = [[0, 1], [2, 3], [4, 5], [6, 7]]

    with tile.TileContext(nc) as tc:
        with (
            tc.tile_pool(name="const", bufs=1) as const,
            tc.tile_pool(name="proj", bufs=1) as proj,
            tc.tile_pool(name="ppsum", bufs=3, space="PSUM") as ppsum,
            tc.tile_pool(name="opsum_a", bufs=2, space="PSUM") as opsum_a,
            tc.tile_pool(name="opsum_b", bufs=1, space="PSUM") as opsum_b,
            tc.tile_pool(name="spsum", bufs=2, space="PSUM") as spsum,
        ):
            ones_sb = const.tile([P, 1], F32)
            nc.any.memset(ones_sb[:], 1.0)
            # biases ride the (otherwise idle-at-start) gpsimd queue so the
            # HWDGE queues carry only the big input transfers
            bq_sb = const.tile([P, FO], F32)
            nc.gpsimd.dma_start(out=bq_sb[:], in_=bq_d[:])
            bk_sb = const.tile([P, FO], F32)
            nc.gpsimd.dma_start(out=bk_sb[:], in_=bk_d[:])
            # p-state warm-up fodder: the PE clock ramps for ~3-4us after an
            # idle stretch, so while the first inputs stream in we keep the
            # PE busy on throwaway matmuls and enter the first real chain at
            # full clock.
            junk_l = const.tile([P, P], BF16)
            nc.any.memset(junk_l[:], 0.0)
            junk_r = const.tile([P, 512], BF16)
            nc.any.memset(junk_r[:], 0.0)

            # The k-proj gating inputs live in the outermost pool so their
            # DMAs issue before the io-pool-entry barrier (~1us earlier than
            # the rest of the priming).
            kt_in = const.tile([P, KB, EO, 512], BF16)
            wk_sb = const.tile([P, FO, EO, P], BF16)
            nc.sync.dma_start(out=kt_in[:, 0, :, 0:256],
                              in_=kt_d[:, 0, :, 0:256])
            for fp in range(4):
                nc.sync.dma_start(
                    out=wk_sb[:, 2 * fp:2 * fp + 2],
                    in_=wkt_d[:, 2 * fp:2 * fp + 2])
            nc.sync.dma_start(out=kt_in[:, 0, :, 256:512],
                              in_=kt_d[:, 0, :, 256:512])
            nc.sync.dma_start(out=kt_in[:, 1], in_=kt_d[:, 1])

            # persistent projected tensors.  kt_proj / the gathered half of
            # v_proj are written ONLY by the collective readbacks.
            qt_proj = proj.tile([P, FO, SQ], BF16)   # [f-inner, fo, q]
            kt_proj = proj.tile([P, FO, S], BF16)    # [f-inner, fo, k]
            v_proj = proj.tile([P, KC, E], BF16)     # [k-inner, kc, f]

            # ---------------- phase A: projections ----------------
            with tc.tile_pool(name="io", bufs=1) as io:
                # ALL input priming on the sync HWDGE queue, in exact
                # consumption order — one queue at full HBM share, FIFO, so
                # each chunk lands just ahead of the chain that consumes it.
                # (Splitting across queues makes the queues COMPETE for HBM
                # bandwidth and the gating chunk lands later, which stalls
                # chains and drops the PE p-state — measured, not theory.)
                vt_all = io.tile([P, KC // 2, EO, P], BF16)
                nc.sync.dma_start(out=vt_all[:], in_=vt_d[:])
                wv_sb = io.tile([P, EO, E], BF16)
                nc.sync.dma_start(out=wv_sb[:], in_=wvt_d[:])
                wq_sb = io.tile([P, FO, EO, P], BF16)
                nc.sync.dma_start(out=wq_sb[:], in_=wqt_d[:])
                qt_in = io.tile([P, QB, EO, 512], BF16)
                nc.sync.dma_start(out=qt_in[:], in_=qt_d[:])

                # PE p-state warm-up while the first inputs stream in
                # (throwaway matmuls; results discarded).  Sized so the ramp
                # completes (~3us) and the last one ends right as the first
                # real chain's inputs land.
                for wi in range(2):
                    ps = ppsum.tile([P, 512], F32, tag="pp")
                    for _ in range(4):
                        nc.tensor.matmul(ps[:], junk_l[:], junk_r[:],
                                         start=True, stop=True)

                # k projection, own 1024-row half, kb-outer.  Chains evict
                # into a contiguous staging tile so each kb half goes to the
                # collective input as ONE contiguous 1MB DMA.  Staging rides
                # the scalar HWDGE queue (fast, and idle once wk is in);
                # the gpsimd queue only carries the AllGather doorbells.
                kstage = io.tile([P, KB, FO, 512], BF16)
                # kb0 runs as 256-wide chains: the first pass needs only the
                # first 256 kt columns (0.5MB) + the wk pair chunks, so real
                # work starts ~2us earlier while the rest streams in.
                for h in range(2):
                    for fo in range(FO):
                        ps = ppsum.tile([P, 512], F32, tag="pp")
                        for eo in range(EO):
                            nc.tensor.matmul(
                                ps[:, 0:256],
                                wk_sb[:, fo, eo, :],
                                kt_in[:, 0, eo, h * 256:(h + 1) * 256],
                                start=(eo == 0),
                                stop=(eo == EO - 1),
                            )
                        nc.scalar.activation(
                            kstage[:, 0, fo, h * 256:(h + 1) * 256],
                            ps[:, 0:256],
                            mybir.ActivationFunctionType.Identity,
                            bias=bk_sb[:, fo:fo + 1],
                        )
                nc.scalar.dma_start(out=cck_in[0][:], in_=kstage[:, 0])
                nc.gpsimd.collective_compute(
                    "AllGather",
                    mybir.AluOpType.bypass,
                    replica_groups=REPLICA_GROUPS,
                    ins=[cck_in[0][:]],
                    outs=[cck_out[0][:]],
                )
                for fo in range(FO):
                    ps = ppsum.tile([P, 512], F32, tag="pp")
                    for eo in range(EO):
                        nc.tensor.matmul(
                            ps[:],
                            wk_sb[:, fo, eo, :],
                            kt_in[:, 1, eo, :],
                            start=(eo == 0),
                            stop=(eo == EO - 1),
                        )
                    nc.scalar.activation(
                        kstage[:, 1, fo, :],
                        ps[:],
                        mybir.ActivationFunctionType.Identity,
                        bias=bk_sb[:, fo:fo + 1],
                    )
                nc.scalar.dma_start(out=cck_in[1][:], in_=kstage[:, 1])
                nc.gpsimd.collective_compute(
                    "AllGather",
                    mybir.AluOpType.bypass,
                    replica_groups=REPLICA_GROUPS,
                    ins=[cck_in[1][:]],
                    outs=[cck_out[1][:]],
                )

                # v projection, own half; evicts into v_proj[:, :KC//2, :].
                # Staged + exchanged in two 1MB halves so the first AllGather
                # launches ~14us earlier and each transfer stays small.
                for vh in range(2):
                    for kc in range(vh * (KC // 4), (vh + 1) * (KC // 4)):
                        for fb in range(2):
                            ps = ppsum.tile([P, 512], F32, tag="pp")
                            for eo in range(EO):
                                nc.tensor.matmul(
                                    ps[:],
                                    vt_all[:, kc, eo, :],
                                    wv_sb[:, eo, fb * 512:(fb + 1) * 512],
                                    start=(eo == 0),
                                    stop=(eo == EO - 1),
                                )
                            nc.vector.tensor_copy(
                                out=v_proj[:, kc, fb * 512:(fb + 1) * 512],
                                in_=ps[:],
                            )
                    nc.scalar.dma_start(
                        out=ccv_in[vh][:],
                        in_=v_proj[:, vh * (KC // 4):(vh + 1) * (KC // 4), :])
                    nc.gpsimd.collective_compute(
                        "AllGather",
                        mybir.AluOpType.bypass,
                        replica_groups=REPLICA_GROUPS,
                        ins=[ccv_in[vh][:]],
                        outs=[ccv_out[vh][:]],
                    )

                # q projection: psum[f128, q512] = sum_eo WqT[e,f].T @ qT[e,q]
                for fo in range(FO):
                    for qb in range(QB):
                        ps = ppsum.tile([P, 512], F32, tag="pp")
                        for eo in range(EO):
                            nc.tensor.matmul(
                                ps[:],
                                wq_sb[:, fo, eo, :],
                                qt_in[:, qb, eo, :],
                                start=(eo == 0),
                                stop=(eo == EO - 1),
                            )
                        nc.scalar.activation(
                            qt_proj[:, fo, qb * 512:(qb + 1) * 512],
                            ps[:],
                            mybir.ActivationFunctionType.Identity,
                            bias=bq_sb[:, fo:fo + 1],
                        )

                # Readbacks on the sync queue (behind the priming FIFO, which
                # has drained by the time each AllGather completes).  Local
                # halves too: rank order keeps the program SPMD-uniform.
                for kb in range(KB):
                    for r in range(2):
                        nc.sync.dma_start(
                            out=kt_proj[:, :,
                                        r * SQ + kb * 512:r * SQ + (kb + 1) * 512],
                            in_=cck_out[kb][r])
                if with_mask:
                    pass  # mask DMA issued in phase B (below), before v rb
                for vh in range(2):
                    for r in range(2):
                        nc.sync.dma_start(
                            out=v_proj[:, r * (KC // 2) + vh * (KC // 4):
                                       r * (KC // 2) + (vh + 1) * (KC // 4), :],
                            in_=ccv_out[vh][r])

            # ---------------- phase B: attention ----------------
            with (
                tc.tile_pool(name="phb", bufs=2) as phb,
                tc.tile_pool(name="outp", bufs=3) as outp,
                tc.tile_pool(name="rpool", bufs=8) as rpool,
            ):
                if with_mask:
                    mask_sb = phb.tile([P, KC, SQ], BF16, tag="mask", bufs=1)
                    nc.scalar.dma_start(out=mask_sb[:], in_=mask_d[:])

                # kc slots in collective-readback readiness order (the first
                # kb-half AllGather delivers slots {0-3, 8-11}).  With the
                # early staging this is normally all resident anyway; the
                # order just maximizes slack if a collective runs long.
                KC_ORDER = [0, 1, 2, 3, 8, 9, 10, 11, 4, 5, 6, 7, 12, 13, 14, 15]
                expTs, recips = [], []
                for qb in range(QB):
                    # scores + exp for this q 512-block
                    expT = phb.tile([P, KC, 512], BF16, tag="expT")
                    expTs.append(expT)
                    sums_acc = phb.tile([P, 512], F32, tag="sumacc")
                    for ki, kc in enumerate(KC_ORDER):
                        ps = ppsum.tile([P, 512], F32, tag="pp")
                        for fo in range(FO):
                            nc.tensor.matmul(
                                ps[:],
                                kt_proj[:, fo, kc * P:(kc + 1) * P],
                                qt_proj[:, fo, qb * 512:(qb + 1) * 512],
                                start=(fo == 0),
                                stop=(fo == FO - 1),
                            )
                        if with_mask:
                            nc.vector.tensor_scalar_mul(ps[:], ps[:], INV_SCALE)
                            nc.vector.tensor_add(
                                ps[:], ps[:],
                                mask_sb[:, kc, qb * 512:(qb + 1) * 512],
                            )
                            nc.scalar.activation(
                                expT[:, kc, :], ps[:],
                                mybir.ActivationFunctionType.Exp,
                            )
                        else:
                            nc.scalar.activation(
                                expT[:, kc, :], ps[:],
                                mybir.ActivationFunctionType.Exp,
                                scale=INV_SCALE,
                            )
                        # accumulate softmax denominators on DVE (frees the
                        # PE from the ones-column matmuls)
                        if ki == 0:
                            nc.vector.tensor_copy(
                                out=sums_acc[:], in_=expT[:, kc, :])
                        else:
                            nc.vector.tensor_add(
                                sums_acc[:], sums_acc[:], expT[:, kc, :])
                    # fold the partition axis with tiny fp32 ones-matmuls:
                    # psum[q128, 1] = sums_acc[:, qslice].T @ ones — lands
                    # directly in the per-partition layout the out-evict
                    # scale needs
                    qb_recips = []
                    for qi in range(4):
                        pf = spsum.tile([P, 1], F32, tag="pf")
                        nc.tensor.matmul(
                            pf[:],
                            sums_acc[:, qi * P:(qi + 1) * P],
                            ones_sb[:],
                            start=True, stop=True,
                        )
                        rt = rpool.tile([P, 1], F32, tag="recip")
                        nc.vector.reciprocal(rt[:], pf[:])
                        qb_recips.append(rt)
                    recips.append(qb_recips)

                # attn @ V per 128-row q tile.  pa chain (f 0:512) runs
                # fully, evicts, and stores its half while the pb chain
                # runs — halves the post-last-matmul tail.
                for qb in range(QB):
                    expT = expTs[qb]
                    for qi in range(4):
                        qg = qb * 4 + qi
                        rt = recips[qb][qi]
                        out_sb = outp.tile([P, E], BF16, tag="outsb")
                        pa = opsum_a.tile([P, 512], F32, tag="pa")
                        for ki, kc in enumerate(KC_ORDER):
                            nc.tensor.matmul(
                                pa[:], expT[:, kc, qi * P:(qi + 1) * P],
                                v_proj[:, kc, 0:512],
                                start=(ki == 0), stop=(ki == KC - 1))
                        nc.scalar.activation(
                            out_sb[:, 0:512], pa[:],
                            mybir.ActivationFunctionType.Copy,
                            scale=rt[:],
                        )
                        nc.sync.dma_start(
                            out=out_d[:, qg, 0:512], in_=out_sb[:, 0:512])
                        if qg == QB * 4 - 1:
                            # very last tile: the f 512:1024 half runs as two
                            # 256-wide chains so the first eviction + store
                            # overlap the second chain, shrinking the tail
                            for hb in range(2):
                                pool = opsum_a if hb == 0 else opsum_b
                                pbh = pool.tile([P, 512], F32,
                                                tag="pa" if hb == 0 else "pb")
                                sl = slice(512 + hb * 256, 768 + hb * 256)
                                for ki, kc in enumerate(KC_ORDER):
                                    nc.tensor.matmul(
                                        pbh[:, 0:256],
                                        expT[:, kc, qi * P:(qi + 1) * P],
                                        v_proj[:, kc, sl],
                                        start=(ki == 0), stop=(ki == KC - 1))
                                nc.scalar.activation(
                                    out_sb[:, sl], pbh[:, 0:256],
                                    mybir.ActivationFunctionType.Copy,
                                    scale=rt[:],
                                )
                                nc.sync.dma_start(
                                    out=out_d[:, qg, sl], in_=out_sb[:, sl])
                        else:
                            pb = opsum_b.tile([P, 512], F32, tag="pb")
                            for ki, kc in enumerate(KC_ORDER):
                                nc.tensor.matmul(
                                    pb[:], expT[:, kc, qi * P:(qi + 1) * P],
                                    v_proj[:, kc, 512:1024],
                                    start=(ki == 0), stop=(ki == KC - 1))
                            nc.scalar.activation(
                                out_sb[:, 512:1024], pb[:],
                                mybir.ActivationFunctionType.Copy,
                                scale=rt[:],
                            )
                            nc.sync.dma_start(
                                out=out_d[:, qg, 512:1024],
                                in_=out_sb[:, 512:1024])

    nc.compile()
    return nc


def _bf16_kb_tiled(x, nblk):
    """[R(e), C(s)] fp32 -> [128, nblk, R//128, C//nblk] bf16.

    partition = inner e index; axis1 = 512-col block of s; axis2 = e-outer.
    """
    r, c = x.shape
    w = c // nblk
    return (
        np.ascontiguousarray(x).astype(ml_dtypes.bfloat16)
        .reshape(r // P, P, nblk, w).transpose(1, 2, 0, 3).copy()
    )


def _prepare_in_maps(query, key, value, attn_mask, Wq, bq, Wk, bk, Wv, bv,
                     with_mask):
    query = np.asarray(query, np.float32)
    key = np.asarray(key, np.float32)
    value = np.asarray(value, np.float32)
    w_t = {}
    for name, w in (("wqt", Wq), ("wkt", Wk)):
        # W.T is [e(in), f(out)] -> [P(e-inner), FO, EO, 128(f-inner)]
        wT = np.ascontiguousarray(np.asarray(w, np.float32).T)
        w_t[name] = (
            wT.astype(ml_dtypes.bfloat16)
            .reshape(EO, P, FO, P).transpose(1, 2, 0, 3).copy()
        )
    wvT = np.ascontiguousarray(np.asarray(Wv, np.float32).T)
    w_t["wvt"] = (
        wvT.astype(ml_dtypes.bfloat16)
        .reshape(EO, P, E).transpose(1, 0, 2).copy()
    )
    bq_t = np.asarray(bq, np.float32).reshape(FO, P).T.copy()
    bk_t = np.asarray(bk, np.float32).reshape(FO, P).T.copy()

    in_maps = []
    for c in range(N_CORES):
        b, h = c // 2, c % 2
        qt = _bf16_kb_tiled(query[b, h * SQ:(h + 1) * SQ, :].T, QB)
        kt = _bf16_kb_tiled(key[b].T[:, h * SQ:(h + 1) * SQ], KB)
        # vt slabs for own k-half: [p(e-inner), kc_local, eo, w(k-inner)]
        vt = (
            np.ascontiguousarray(value[b].T[:, h * SQ:(h + 1) * SQ])
            .astype(ml_dtypes.bfloat16)
            .reshape(EO, P, KC // 2, P).transpose(1, 2, 0, 3).copy()
        )
        m = dict(qt=qt, kt=kt, vt=vt, bq=bq_t, bk=bk_t, **w_t)
        if with_mask:
            mt = np.asarray(attn_mask[b, h * SQ:(h + 1) * SQ, :], np.float32).T
            m["maskt"] = (
                mt.astype(ml_dtypes.bfloat16)
                .reshape(KC, P, SQ).transpose(1, 0, 2).copy()
            )
        in_maps.append(m)
    return in_maps


def _run(inputs, trace=False):
    with_mask = bool(np.any(np.asarray(inputs["attn_mask"])))
    key = with_mask
    if key not in _BUILD_CACHE:
        _BUILD_CACHE[key] = _build(with_mask)
    nc = _BUILD_CACHE[key]

    in_maps = _prepare_in_maps(with_mask=with_mask, **inputs)
    res = run_bass_kernel_spmd(nc, in_maps, core_ids=list(range(N_CORES)),
                               trace=trace)

    bv = np.asarray(inputs["bv"], np.float32)
    out = np.zeros((B, S, E), np.float32)
    for c in range(N_CORES):
        b, h = c // 2, c % 2
        oc = np.asarray(res.results[c]["out"], np.float32)  # [P, SQ//P, E]
        out[b, h * SQ:(h + 1) * SQ, :] = (
            oc.transpose(1, 0, 2).reshape(SQ, E) + bv[None, :]
        )
    return out, res


def kernel(**inputs) -> np.ndarray:
    out, _ = _run(inputs, trace=False)
    return out


# revision 27
# speedup vs baseline: 1.0092x; 1.0092x over previous
"""Self-contained Trainium2 Bass kernel for nn_AttentionModel (B=4, S=2048, E=1024).

Model: q/k/v linear projections + scaled-dot-product attention (scale = sqrt(E)).

Sharding (8 NeuronCores): core c handles batch b=c//2, query-row half h=c%2
(1024 q rows). k/v projections are split across the core pair (each projects
its own 1024 k-rows) and exchanged with a pair-local AllGather
(replica_groups [[0,1],[2,3],[4,5],[6,7]]); both halves are read back from
the collective output in absolute rank order, so the program stays fully
SPMD-uniform with no core-dependent addressing.

Device algorithm per core (all matmuls bf16 with fp32 PSUM accumulation):
  qT_proj[f,q]  = WqT.T @ qT_in   (+bq via ACT bias on eviction)
  kT_proj[f,k]  = WkT.T @ kT_in   (+bk)   [own half, then pair AllGather]
  v_proj [k,f]  = vT_in.T @ WvT   [own half, then pair AllGather;
                                   bias bv applied on host after gather]
  scoresT[k,q]  = kT_proj.T @ qT_proj          (per 128k x 512q psum tile)
  expT   [k,q]  = exp(scoresT / sqrt(E))       (ACT, no max-subtraction:
                                                logits are O(+-6), fp32-safe)
  out_un [q,f]  = expT.T @ v_proj              (accumulate over k chunks)
  sums   [q]    = DVE-accumulated exp tiles, partition-folded by a tiny
                  fp32 ones-matmul into per-partition [q,1] layout
  out    [q,f]  = out_un * (1/sums)            (per-partition ACT scale)

Scheduling notes (from perfetto analysis of the previous version):
  - Input priming is split across BOTH HWDGE engines (sync + scalar) so
    descriptor generation and transfers overlap; the k-proj inputs (kt
    halves on scalar, wk fo0-1 first on sync) land first so the PE starts
    ~13us in instead of ~21us.
  - k chains evict into a CONTIGUOUS staging tile (not kt_proj), so each
    kb half is staged to the collective input with ONE contiguous DMA and
    the AllGathers launch much earlier.  kt_proj is written only by the
    collective readbacks.  v staging is one contiguous DMA as well.
  - Readbacks ride the sync queue after priming drains; everything is
    resident long before the consuming phase, which removes the v-readback
    stall + half-rate region that used to sit at the attn@V entry.
  - attn@V runs the pa (f 0:512) chain fully, evicts + stores that half,
    then the pb chain, halving the post-last-matmul tail.

Host pre-tiles every input into fully-contiguous-per-partition bf16 SBUF
layouts, so the device performs no transposes or casts on the inputs.
"""

import sys

for _p in ("/opt/trn_rl_repo", "/root/.axon_site/_ro/trn_rl_repo"):
    if _p not in sys.path:
        sys.path.insert(0, _p)

import numpy as np
import ml_dtypes

import concourse.bacc as bacc
import concourse.mybir as mybir
import concourse.tile as tile
from concourse.bass_utils import run_bass_kernel_spmd

B, S, E = 4, 2048, 1024
P = 128
SQ = S // 2          # q rows per core
N_CORES = 8
EO = E // P          # 8  e-outer chunks
FO = E // P          # 8  f-outer chunks
KC = S // P          # 16 k-row chunks
QB = SQ // 512       # 2  q 512-blocks
KB = SQ // 512       # 2  k 512-blocks (own half)
INV_SCALE = float(1.0 / np.sqrt(np.float32(E)))

BF16 = mybir.dt.bfloat16
F32 = mybir.dt.float32

_BUILD_CACHE: dict = {}


def _build(with_mask: bool):
    nc = bacc.Bacc(
        "TRN2",
        target_bir_lowering=False,
        debug=False,
        enable_asserts=False,
        num_devices=N_CORES,
    )

    # Host-pretiled inputs; every transfer below is contiguous per partition.
    qt_d = nc.declare_dram_parameter("qt", [P, QB, EO, 512], BF16, isOutput=False)
    kt_d = nc.declare_dram_parameter("kt", [P, KB, EO, 512], BF16, isOutput=False)
    vt_d = nc.declare_dram_parameter("vt", [P, KC // 2, EO, P], BF16, isOutput=False)
    wqt_d = nc.declare_dram_parameter("wqt", [P, FO, EO, P], BF16, isOutput=False)
    wkt_d = nc.declare_dram_parameter("wkt", [P, FO, EO, P], BF16, isOutput=False)
    wvt_d = nc.declare_dram_parameter("wvt", [P, EO, E], BF16, isOutput=False)
    bq_d = nc.declare_dram_parameter("bq", [P, FO], F32, isOutput=False)
    bk_d = nc.declare_dram_parameter("bk", [P, FO], F32, isOutput=False)
    if with_mask:
        mask_d = nc.declare_dram_parameter("maskt", [P, KC, SQ], BF16, isOutput=False)
    # bf16 output: the attention output is an attn-weighted average (values
    # ~N(0, 0.03)); bf16 rounding adds ~0.4% relative error on top of the
    # ~0.6% bf16-compute error — far inside the 2e-2 gate — and halves the
    # output DMA traffic + final-store landing time.
    out_d = nc.declare_dram_parameter("out", [P, SQ // P, E], BF16, isOutput=True)

    # pair-AllGather staging buffers (internal DRAM)
    # NOTE: Shared-output collectives need >4-core groups; for 2-core pair
    # groups the output must stay a Local internal tensor. The k exchange is
    # split into two kb-halves so the first AllGather launches early and the
    # CC stream stays busy while phase A continues.
    cck_in = [nc.dram_tensor(f"cck_in{kb}", [P, FO, 512], BF16) for kb in range(KB)]
    cck_out = [nc.dram_tensor(f"cck_out{kb}", [2, P, FO, 512], BF16)
               for kb in range(KB)]
    ccv_in = [nc.dram_tensor(f"ccv_in{h}", [P, KC // 4, E], BF16) for h in range(2)]
    ccv_out = [nc.dram_tensor(f"ccv_out{h}", [2, P, KC // 4, E], BF16)
               for h in range(2)]
    REPLICA_GROUPS = [[0, 1], [2, 3], [4, 5], [6, 7]]

    with tile.TileContext(nc) as tc:
        with (
            tc.tile_pool(name="const", bufs=1) as const,
            tc.tile_pool(name="proj", bufs=1) as proj,
            tc.tile_pool(name="ppsum", bufs=3, space="PSUM") as ppsum,
            tc.tile_pool(name="opsum_a", bufs=2, space="PSUM") as opsum_a,
            tc.tile_pool(name="opsum_b", bufs=1, space="PSUM") as opsum_b,
            tc.tile_pool(name="spsum", bufs=2, space="PSUM") as spsum,
        ):
            ones_sb = const.tile([P, 1], F32)
            nc.any.memset(ones_sb[:], 1.0)
            # biases ride the (otherwise idle-at-start) gpsimd queue so the
            # HWDGE queues carry only the big input transfers
            bq_sb = const.tile([P, FO], F32)
            nc.gpsimd.dma_start(out=bq_sb[:], in_=bq_d[:])
            bk_sb = const.tile([P, FO], F32)
            nc.gpsimd.dma_start(out=bk_sb[:], in_=bk_d[:])
            # p-state warm-up fodder: the PE clock ramps for ~3-4us after an
            # idle stretch, so while the first inputs stream in we keep the
            # PE busy on throwaway matmuls and enter the first real chain at
            # full clock.
            junk_l = const.tile([P, P], BF16)
            nc.any.memset(junk_l[:], 0.0)
            junk_r = const.tile([P, 512], BF16)
            nc.any.memset(junk_r[:], 0.0)

            # The k-proj gating inputs live in the outermost pool so their
            # DMAs issue before the io-pool-entry barrier (~1us earlier than
            # the rest of the priming).
            kt_in = const.tile([P, KB, EO, 512], BF16)
            wk_sb = const.tile([P, FO, EO, P], BF16)
            nc.sync.dma_start(out=kt_in[:, 0, :, 0:256],
                              in_=kt_d[:, 0, :, 0:256])
            for fp in range(4):
                nc.sync.dma_start(
                    out=wk_sb[:, 2 * fp:2 * fp + 2],
                    in_=wkt_d[:, 2 * fp:2 * fp + 2])
            nc.sync.dma_start(out=kt_in[:, 0, :, 256:512],
                              in_=kt_d[:, 0, :, 256:512])
            nc.sync.dma_start(out=kt_in[:, 1], in_=kt_d[:, 1])

            # persistent projected tensors.  kt_proj / the gathered half of
            # v_proj are written ONLY by the collective readbacks.
            qt_proj = proj.tile([P, FO, SQ], BF16)   # [f-inner, fo, q]
            kt_proj = proj.tile([P, FO, S], BF16)    # [f-inner, fo, k]
            v_proj = proj.tile([P, KC, E], BF16)     # [k-inner, kc, f]

            # ---------------- phase A: projections ----------------
            with tc.tile_pool(name="io", bufs=1) as io:
                # ALL input priming on the sync HWDGE queue, in exact
                # consumption order — one queue at full HBM share, FIFO, so
                # each chunk lands just ahead of the chain that consumes it.
                # (Splitting across queues makes the queues COMPETE for HBM
                # bandwidth and the gating chunk lands later, which stalls
                # chains and drops the PE p-state — measured, not theory.)
                vt_all = io.tile([P, KC // 2, EO, P], BF16)
                nc.sync.dma_start(out=vt_all[:], in_=vt_d[:])
                wv_sb = io.tile([P, EO, E], BF16)
                nc.sync.dma_start(out=wv_sb[:], in_=wvt_d[:])
                wq_sb = io.tile([P, FO, EO, P], BF16)
                nc.sync.dma_start(out=wq_sb[:], in_=wqt_d[:])
                qt_in = io.tile([P, QB, EO, 512], BF16)
                nc.sync.dma_start(out=qt_in[:], in_=qt_d[:])

                # PE p-state warm-up while the first inputs stream in
                # (throwaway matmuls; results discarded).  Sized so the ramp
                # completes (~3us) and the last one ends right as the first
                # real chain's inputs land.
                for wi in range(2):
                    ps = ppsum.tile([P, 512], F32, tag="pp")
                    for _ in range(6 - wi):
                        nc.tensor.matmul(ps[:], junk_l[:], junk_r[:],
                                         start=True, stop=True)

                # k projection, own 1024-row half, kb-outer.  Chains evict
                # into a contiguous staging tile so each kb half goes to the
                # collective input as ONE contiguous 1MB DMA.  Staging rides
                # the scalar HWDGE queue (fast, and idle once wk is in);
                # the gpsimd queue only carries the AllGather doorbells.
                kstage = io.tile([P, KB, FO, 512], BF16)
                # kb0 runs as 256-wide chains: the first pass needs only the
                # first 256 kt columns (0.5MB) + the wk pair chunks, so real
                # work starts ~2us earlier while the rest streams in.
                for h in range(2):
                    for fo in range(FO):
                        ps = ppsum.tile([P, 512], F32, tag="pp")
                        for eo in range(EO):
                            nc.tensor.matmul(
                                ps[:, 0:256],
                                wk_sb[:, fo, eo, :],
                                kt_in[:, 0, eo, h * 256:(h + 1) * 256],
                                start=(eo == 0),
                                stop=(eo == EO - 1),
                            )
                        nc.scalar.activation(
                            kstage[:, 0, fo, h * 256:(h + 1) * 256],
                            ps[:, 0:256],
                            mybir.ActivationFunctionType.Identity,
                            bias=bk_sb[:, fo:fo + 1],
                        )
                nc.scalar.dma_start(out=cck_in[0][:], in_=kstage[:, 0])
                nc.gpsimd.collective_compute(
                    "AllGather",
                    mybir.AluOpType.bypass,
                    replica_groups=REPLICA_GROUPS,
                    ins=[cck_in[0][:]],
                    outs=[cck_out[0][:]],
                )
                for fo in range(FO):
                    ps = ppsum.tile([P, 512], F32, tag="pp")
                    for eo in range(EO):
                        nc.tensor.matmul(
                            ps[:],
                            wk_sb[:, fo, eo, :],
                            kt_in[:, 1, eo, :],
                            start=(eo == 0),
                            stop=(eo == EO - 1),
                        )
                    nc.scalar.activation(
                        kstage[:, 1, fo, :],
                        ps[:],
                        mybir.ActivationFunctionType.Identity,
                        bias=bk_sb[:, fo:fo + 1],
                    )
                nc.scalar.dma_start(out=cck_in[1][:], in_=kstage[:, 1])
                nc.gpsimd.collective_compute(
                    "AllGather",
                    mybir.AluOpType.bypass,
                    replica_groups=REPLICA_GROUPS,
                    ins=[cck_in[1][:]],
                    outs=[cck_out[1][:]],
                )

                # v projection, own half; evicts into v_proj[:, :KC//2, :].
                # Staged + exchanged in two 1MB halves so the first AllGather
                # launches ~14us earlier and each transfer stays small.
                for vh in range(2):
                    for kc in range(vh * (KC // 4), (vh + 1) * (KC // 4)):
                        for fb in range(2):
                            ps = ppsum.tile([P, 512], F32, tag="pp")
                            for eo in range(EO):
                                nc.tensor.matmul(
                                    ps[:],
                                    vt_all[:, kc, eo, :],
                                    wv_sb[:, eo, fb * 512:(fb + 1) * 512],
                                    start=(eo == 0),
                                    stop=(eo == EO - 1),
                                )
                            nc.vector.tensor_copy(
                                out=v_proj[:, kc, fb * 512:(fb + 1) * 512],
                                in_=ps[:],
                            )
                    nc.scalar.dma_start(
                        out=ccv_in[vh][:],
                        in_=v_proj[:, vh * (KC // 4):(vh + 1) * (KC // 4), :])
                    nc.gpsimd.collective_compute(
                        "AllGather",
                        mybir.AluOpType.bypass,
                        replica_groups=REPLICA_GROUPS,
                        ins=[ccv_in[vh][:]],
                        outs=[ccv_out[vh][:]],
                    )

                # q projection: psum[f128, q512] = sum_eo WqT[e,f].T @ qT[e,q]
                for fo in range(FO):
                    for qb in range(QB):
                        ps = ppsum.tile([P, 512], F32, tag="pp")
                        for eo in range(EO):
                            nc.tensor.matmul(
                                ps[:],
                                wq_sb[:, fo, eo, :],
                                qt_in[:, qb, eo, :],
                                start=(eo == 0),
                                stop=(eo == EO - 1),
                            )
                        nc.scalar.activation(
                            qt_proj[:, fo, qb * 512:(qb + 1) * 512],
                            ps[:],
                            mybir.ActivationFunctionType.Identity,
                            bias=bq_sb[:, fo:fo + 1],
                        )

                # Readbacks on the sync queue (behind the priming FIFO, which
                # has drained by the time each AllGather completes).  Local
                # halves too: rank order keeps the program SPMD-uniform.
                for kb in range(KB):
                    for r in range(2):
                        nc.sync.dma_start(
                            out=kt_proj[:, :,
                                        r * SQ + kb * 512:r * SQ + (kb + 1) * 512],
                            in_=cck_out[kb][r])
                if with_mask:
                    pass  # mask DMA issued in phase B (below), before v rb
                for vh in range(2):
                    for r in range(2):
                        nc.sync.dma_start(
                            out=v_proj[:, r * (KC // 2) + vh * (KC // 4):
                                       r * (KC // 2) + (vh + 1) * (KC // 4), :],
                            in_=ccv_out[vh][r])

            # ---------------- phase B: attention ----------------
            with (
                tc.tile_pool(name="phb", bufs=2) as phb,
                tc.tile_pool(name="outp", bufs=3) as outp,
                tc.tile_pool(name="rpool", bufs=8) as rpool,
            ):
                if with_mask:
                    mask_sb = phb.tile([P, KC, SQ], BF16, tag="mask", bufs=1)
                    nc.scalar.dma_start(out=mask_sb[:], in_=mask_d[:])

                # kc slots in collective-readback readiness order (the first
                # kb-half AllGather delivers slots {0-3, 8-11}).  With the
                # early staging this is normally all resident anyway; the
                # order just maximizes slack if a collective runs long.
                KC_ORDER = [0, 1, 2, 3, 8, 9, 10, 11, 4, 5, 6, 7, 12, 13, 14, 15]
                expTs, recips = [], []
                for qb in range(QB):
                    # scores + exp for this q 512-block
                    expT = phb.tile([P, KC, 512], BF16, tag="expT")
                    expTs.append(expT)
                    sums_acc = phb.tile([P, 512], F32, tag="sumacc")
                    for ki, kc in enumerate(KC_ORDER):
                        ps = ppsum.tile([P, 512], F32, tag="pp")
                        for fo in range(FO):
                            nc.tensor.matmul(
                                ps[:],
                                kt_proj[:, fo, kc * P:(kc + 1) * P],
                                qt_proj[:, fo, qb * 512:(qb + 1) * 512],
                                start=(fo == 0),
                                stop=(fo == FO - 1),
                            )
                        if with_mask:
                            nc.vector.tensor_scalar_mul(ps[:], ps[:], INV_SCALE)
                            nc.vector.tensor_add(
                                ps[:], ps[:],
                                mask_sb[:, kc, qb * 512:(qb + 1) * 512],
                            )
                            nc.scalar.activation(
                                expT[:, kc, :], ps[:],
                                mybir.ActivationFunctionType.Exp,
                            )
                        else:
                            nc.scalar.activation(
                                expT[:, kc, :], ps[:],
                                mybir.ActivationFunctionType.Exp,
                                scale=INV_SCALE,
                            )
                        # accumulate softmax denominators on DVE (frees the
                        # PE from the ones-column matmuls)
                        if ki == 0:
                            nc.vector.tensor_copy(
                                out=sums_acc[:], in_=expT[:, kc, :])
                        else:
                            nc.vector.tensor_add(
                                sums_acc[:], sums_acc[:], expT[:, kc, :])
                    # fold the partition axis with tiny fp32 ones-matmuls:
                    # psum[q128, 1] = sums_acc[:, qslice].T @ ones — lands
                    # directly in the per-partition layout the out-evict
                    # scale needs
                    qb_recips = []
                    for qi in range(4):
                        pf = spsum.tile([P, 1], F32, tag="pf")
                        nc.tensor.matmul(
                            pf[:],
                            sums_acc[:, qi * P:(qi + 1) * P],
                            ones_sb[:],
                            start=True, stop=True,
                        )
                        rt = rpool.tile([P, 1], F32, tag="recip")
                        nc.vector.reciprocal(rt[:], pf[:])
                        qb_recips.append(rt)
                    recips.append(qb_recips)

                # attn @ V per 128-row q tile.  pa chain (f 0:512) runs
                # fully, evicts, and stores its half while the pb chain
                # runs — halves the post-last-matmul tail.
                for qb in range(QB):
                    expT = expTs[qb]
                    for qi in range(4):
                        qg = qb * 4 + qi
                        rt = recips[qb][qi]
                        out_sb = outp.tile([P, E], BF16, tag="outsb")
                        pa = opsum_a.tile([P, 512], F32, tag="pa")
                        for ki, kc in enumerate(KC_ORDER):
                            nc.tensor.matmul(
                                pa[:], expT[:, kc, qi * P:(qi + 1) * P],
                                v_proj[:, kc, 0:512],
                                start=(ki == 0), stop=(ki == KC - 1))
                        nc.scalar.activation(
                            out_sb[:, 0:512], pa[:],
                            mybir.ActivationFunctionType.Copy,
                            scale=rt[:],
                        )
                        nc.sync.dma_start(
                            out=out_d[:, qg, 0:512], in_=out_sb[:, 0:512])
                        if qg == QB * 4 - 1:
                            # very last tile: the f 512:1024 half runs as two
                            # 256-wide chains so the first eviction + store
                            # overlap the second chain, shrinking the tail
                            for hb in range(2):
                                pool = opsum_a if hb == 0 else opsum_b
                                pbh = pool.tile([P, 512], F32,
                                                tag="pa" if hb == 0 else "pb")
                                sl = slice(512 + hb * 256, 768 + hb * 256)
                                for ki, kc in enumerate(KC_ORDER):
                                    nc.tensor.matmul(
                                        pbh[:, 0:256],
                                        expT[:, kc, qi * P:(qi + 1) * P],
                                        v_proj[:, kc, sl],
                                        start=(ki == 0), stop=(ki == KC - 1))
                                nc.scalar.activation(
                                    out_sb[:, sl], pbh[:, 0:256],
                                    mybir.ActivationFunctionType.Copy,
                                    scale=rt[:],
                                )
                                nc.sync.dma_start(
                                    out=out_d[:, qg, sl], in_=out_sb[:, sl])
                        else:
                            pb = opsum_b.tile([P, 512], F32, tag="pb")
                            for ki, kc in enumerate(KC_ORDER):
                                nc.tensor.matmul(
                                    pb[:], expT[:, kc, qi * P:(qi + 1) * P],
                                    v_proj[:, kc, 512:1024],
                                    start=(ki == 0), stop=(ki == KC - 1))
                            nc.scalar.activation(
                                out_sb[:, 512:1024], pb[:],
                                mybir.ActivationFunctionType.Copy,
                                scale=rt[:],
                            )
                            nc.sync.dma_start(
                                out=out_d[:, qg, 512:1024],
                                in_=out_sb[:, 512:1024])

    nc.compile()
    return nc


def _bf16_kb_tiled(x, nblk):
    """[R(e), C(s)] fp32 -> [128, nblk, R//128, C//nblk] bf16.

    partition = inner e index; axis1 = 512-col block of s; axis2 = e-outer.
    """
    r, c = x.shape
    w = c // nblk
    return (
        np.ascontiguousarray(x).astype(ml_dtypes.bfloat16)
        .reshape(r // P, P, nblk, w).transpose(1, 2, 0, 3).copy()
    )


def _prepare_in_maps(query, key, value, attn_mask, Wq, bq, Wk, bk, Wv, bv,
                     with_mask):
    query = np.asarray(query, np.float32)
    key = np.asarray(key, np.float32)
    value = np.asarray(value, np.float32)
    w_t = {}
    for name, w in (("wqt", Wq), ("wkt", Wk)):
        # W.T is [e(in), f(out)] -> [P(e-inner), FO, EO, 128(f-inner)]
        wT = np.ascontiguousarray(np.asarray(w, np.float32).T)
        w_t[name] = (
            wT.astype(ml_dtypes.bfloat16)
            .reshape(EO, P, FO, P).transpose(1, 2, 0, 3).copy()
        )
    wvT = np.ascontiguousarray(np.asarray(Wv, np.float32).T)
    w_t["wvt"] = (
        wvT.astype(ml_dtypes.bfloat16)
        .reshape(EO, P, E).transpose(1, 0, 2).copy()
    )
    bq_t = np.asarray(bq, np.float32).reshape(FO, P).T.copy()
    bk_t = np.asarray(bk, np.float32).reshape(FO, P).T.copy()

    in_maps = []
    for c in range(N_CORES):
        b, h = c // 2, c % 2
        qt = _bf16_kb_tiled(query[b, h * SQ:(h + 1) * SQ, :].T, QB)
        kt = _bf16_kb_tiled(key[b].T[:, h * SQ:(h + 1) * SQ], KB)
        # vt slabs for own k-half: [p(e-inner), kc_local, eo, w(k-inner)]
        vt = (
            np.ascontiguousarray(value[b].T[:, h * SQ:(h + 1) * SQ])
            .astype(ml_dtypes.bfloat16)
            .reshape(EO, P, KC // 2, P).transpose(1, 2, 0, 3).copy()
        )
        m = dict(qt=qt, kt=kt, vt=vt, bq=bq_t, bk=bk_t, **w_t)
        if with_mask:
            mt = np.asarray(attn_mask[b, h * SQ:(h + 1) * SQ, :], np.float32).T
            m["maskt"] = (
                mt.astype(ml_dtypes.bfloat16)
                .reshape(KC, P, SQ).transpose(1, 0, 2).copy()
            )
        in_maps.append(m)
    return in_maps


def _run(inputs, trace=False):
    with_mask = bool(np.any(np.asarray(inputs["attn_mask"])))
    key = with_mask
    if key not in _BUILD_CACHE:
        _BUILD_CACHE[key] = _build(with_mask)
    nc = _BUILD_CACHE[key]

    in_maps = _prepare_in_maps(with_mask=with_mask, **inputs)
    res = run_bass_kernel_spmd(nc, in_maps, core_ids=list(range(N_CORES)),
                               trace=trace)

    bv = np.asarray(inputs["bv"], np.float32)
    out = np.zeros((B, S, E), np.float32)
    for c in range(N_CORES):
        b, h = c // 2, c % 2
        oc = np.asarray(res.results[c]["out"], np.float32)  # [P, SQ//P, E]
        out[b, h * SQ:(h + 1) * SQ, :] = (
            oc.transpose(1, 0, 2).reshape(SQ, E) + bv[None, :]
        )
    return out, res


def kernel(**inputs) -> np.ndarray:
    out, _ = _run(inputs, trace=False)
    return out


# revision 29
# speedup vs baseline: 1.0095x; 1.0002x over previous
"""Self-contained Trainium2 Bass kernel for nn_AttentionModel (B=4, S=2048, E=1024).

Model: q/k/v linear projections + scaled-dot-product attention (scale = sqrt(E)).

Sharding (8 NeuronCores): core c handles batch b=c//2, query-row half h=c%2
(1024 q rows). k/v projections are split across the core pair (each projects
its own 1024 k-rows) and exchanged with a pair-local AllGather
(replica_groups [[0,1],[2,3],[4,5],[6,7]]); both halves are read back from
the collective output in absolute rank order, so the program stays fully
SPMD-uniform with no core-dependent addressing.

Device algorithm per core (all matmuls bf16 with fp32 PSUM accumulation):
  qT_proj[f,q]  = WqT.T @ qT_in   (+bq via ACT bias on eviction)
  kT_proj[f,k]  = WkT.T @ kT_in   (+bk)   [own half, then pair AllGather]
  v_proj [k,f]  = vT_in.T @ WvT   [own half, then pair AllGather;
                                   bias bv applied on host after gather]
  scoresT[k,q]  = kT_proj.T @ qT_proj          (per 128k x 512q psum tile)
  expT   [k,q]  = exp(scoresT / sqrt(E))       (ACT, no max-subtraction:
                                                logits are O(+-6), fp32-safe)
  out_un [q,f]  = expT.T @ v_proj              (accumulate over k chunks)
  sums   [q]    = DVE-accumulated exp tiles, partition-folded by a tiny
                  fp32 ones-matmul into per-partition [q,1] layout
  out    [q,f]  = out_un * (1/sums)            (per-partition ACT scale)

Scheduling notes (from perfetto analysis of the previous version):
  - Input priming is split across BOTH HWDGE engines (sync + scalar) so
    descriptor generation and transfers overlap; the k-proj inputs (kt
    halves on scalar, wk fo0-1 first on sync) land first so the PE starts
    ~13us in instead of ~21us.
  - k chains evict into a CONTIGUOUS staging tile (not kt_proj), so each
    kb half is staged to the collective input with ONE contiguous DMA and
    the AllGathers launch much earlier.  kt_proj is written only by the
    collective readbacks.  v staging is one contiguous DMA as well.
  - Readbacks ride the sync queue after priming drains; everything is
    resident long before the consuming phase, which removes the v-readback
    stall + half-rate region that used to sit at the attn@V entry.
  - attn@V runs the pa (f 0:512) chain fully, evicts + stores that half,
    then the pb chain, halving the post-last-matmul tail.

Host pre-tiles every input into fully-contiguous-per-partition bf16 SBUF
layouts, so the device performs no transposes or casts on the inputs.
"""

import sys

for _p in ("/opt/trn_rl_repo", "/root/.axon_site/_ro/trn_rl_repo"):
    if _p not in sys.path:
        sys.path.insert(0, _p)

import numpy as np
import ml_dtypes

import concourse.bacc as bacc
import concourse.mybir as mybir
import concourse.tile as tile
from concourse.bass_utils import run_bass_kernel_spmd

B, S, E = 4, 2048, 1024
P = 128
SQ = S // 2          # q rows per core
N_CORES = 8
EO = E // P          # 8  e-outer chunks
FO = E // P          # 8  f-outer chunks
KC = S // P          # 16 k-row chunks
QB = SQ // 512       # 2  q 512-blocks
KB = SQ // 512       # 2  k 512-blocks (own half)
INV_SCALE = float(1.0 / np.sqrt(np.float32(E)))

BF16 = mybir.dt.bfloat16
F32 = mybir.dt.float32

_BUILD_CACHE: dict = {}


def _build(with_mask: bool):
    nc = bacc.Bacc(
        "TRN2",
        target_bir_lowering=False,
        debug=False,
        enable_asserts=False,
        num_devices=N_CORES,
    )

    # Host-pretiled inputs; every transfer below is contiguous per partition.
    qt_d = nc.declare_dram_parameter("qt", [P, QB, EO, 512], BF16, isOutput=False)
    kt_d = nc.declare_dram_parameter("kt", [P, KB, EO, 512], BF16, isOutput=False)
    vt_d = nc.declare_dram_parameter("vt", [P, KC // 2, EO, P], BF16, isOutput=False)
    wqt_d = nc.declare_dram_parameter("wqt", [P, FO, EO, P], BF16, isOutput=False)
    wkt_d = nc.declare_dram_parameter("wkt", [P, FO, EO, P], BF16, isOutput=False)
    wvt_d = nc.declare_dram_parameter("wvt", [P, EO, E], BF16, isOutput=False)
    bq_d = nc.declare_dram_parameter("bq", [P, FO], F32, isOutput=False)
    bk_d = nc.declare_dram_parameter("bk", [P, FO], F32, isOutput=False)
    if with_mask:
        mask_d = nc.declare_dram_parameter("maskt", [P, KC, SQ], BF16, isOutput=False)
    # bf16 output: the attention output is an attn-weighted average (values
    # ~N(0, 0.03)); bf16 rounding adds ~0.4% relative error on top of the
    # ~0.6% bf16-compute error — far inside the 2e-2 gate — and halves the
    # output DMA traffic + final-store landing time.
    out_d = nc.declare_dram_parameter("out", [P, SQ // P, E], BF16, isOutput=True)

    # pair-AllGather staging buffers (internal DRAM)
    # NOTE: Shared-output collectives need >4-core groups; for 2-core pair
    # groups the output must stay a Local internal tensor. The k exchange is
    # split into two kb-halves so the first AllGather launches early and the
    # CC stream stays busy while phase A continues.
    cck_in = [nc.dram_tensor(f"cck_in{kb}", [P, FO, 512], BF16) for kb in range(KB)]
    cck_out = [nc.dram_tensor(f"cck_out{kb}", [2, P, FO, 512], BF16)
               for kb in range(KB)]
    ccv_in = [nc.dram_tensor(f"ccv_in{h}", [P, KC // 4, E], BF16) for h in range(2)]
    ccv_out = [nc.dram_tensor(f"ccv_out{h}", [2, P, KC // 4, E], BF16)
               for h in range(2)]
    REPLICA_GROUPS = [[0, 1], [2, 3], [4, 5], [6, 7]]

    with tile.TileContext(nc) as tc:
        with (
            tc.tile_pool(name="const", bufs=1) as const,
            tc.tile_pool(name="proj", bufs=1) as proj,
            tc.tile_pool(name="ppsum", bufs=3, space="PSUM") as ppsum,
            tc.tile_pool(name="opsum_a", bufs=2, space="PSUM") as opsum_a,
            tc.tile_pool(name="opsum_b", bufs=1, space="PSUM") as opsum_b,
            tc.tile_pool(name="spsum", bufs=2, space="PSUM") as spsum,
        ):
            ones_sb = const.tile([P, 1], F32)
            nc.any.memset(ones_sb[:], 1.0)
            # biases ride the (otherwise idle-at-start) gpsimd queue so the
            # HWDGE queues carry only the big input transfers
            bq_sb = const.tile([P, FO], F32)
            nc.gpsimd.dma_start(out=bq_sb[:], in_=bq_d[:])
            bk_sb = const.tile([P, FO], F32)
            nc.gpsimd.dma_start(out=bk_sb[:], in_=bk_d[:])
            # p-state warm-up fodder: the PE clock ramps for ~3-4us after an
            # idle stretch, so while the first inputs stream in we keep the
            # PE busy on throwaway matmuls and enter the first real chain at
            # full clock.
            junk_l = const.tile([P, P], BF16)
            nc.any.memset(junk_l[:], 0.0)
            junk_r = const.tile([P, 512], BF16)
            nc.any.memset(junk_r[:], 0.0)

            # The k-proj gating inputs live in the outermost pool so their
            # DMAs issue before the io-pool-entry barrier (~1us earlier than
            # the rest of the priming).
            kt_in = const.tile([P, KB, EO, 512], BF16)
            wk_sb = const.tile([P, FO, EO, P], BF16)
            nc.sync.dma_start(out=kt_in[:, 0, :, 0:256],
                              in_=kt_d[:, 0, :, 0:256])
            nc.sync.dma_start(out=wk_sb[:, 0:1], in_=wkt_d[:, 0:1])
            nc.sync.dma_start(out=wk_sb[:, 1:2], in_=wkt_d[:, 1:2])
            for fp in range(1, 4):
                nc.sync.dma_start(
                    out=wk_sb[:, 2 * fp:2 * fp + 2],
                    in_=wkt_d[:, 2 * fp:2 * fp + 2])
            nc.sync.dma_start(out=kt_in[:, 0, :, 256:512],
                              in_=kt_d[:, 0, :, 256:512])
            nc.sync.dma_start(out=kt_in[:, 1], in_=kt_d[:, 1])

            # persistent projected tensors.  kt_proj / the gathered half of
            # v_proj are written ONLY by the collective readbacks.
            qt_proj = proj.tile([P, FO, SQ], BF16)   # [f-inner, fo, q]
            kt_proj = proj.tile([P, FO, S], BF16)    # [f-inner, fo, k]
            v_proj = proj.tile([P, KC, E], BF16)     # [k-inner, kc, f]

            # ---------------- phase A: projections ----------------
            with tc.tile_pool(name="io", bufs=1) as io:
                # ALL input priming on the sync HWDGE queue, in exact
                # consumption order — one queue at full HBM share, FIFO, so
                # each chunk lands just ahead of the chain that consumes it.
                # (Splitting across queues makes the queues COMPETE for HBM
                # bandwidth and the gating chunk lands later, which stalls
                # chains and drops the PE p-state — measured, not theory.)
                vt_all = io.tile([P, KC // 2, EO, P], BF16)
                nc.sync.dma_start(out=vt_all[:], in_=vt_d[:])
                wv_sb = io.tile([P, EO, E], BF16)
                nc.sync.dma_start(out=wv_sb[:], in_=wvt_d[:])
                wq_sb = io.tile([P, FO, EO, P], BF16)
                nc.sync.dma_start(out=wq_sb[:], in_=wqt_d[:])
                qt_in = io.tile([P, QB, EO, 512], BF16)
                nc.sync.dma_start(out=qt_in[:], in_=qt_d[:])

                # PE p-state warm-up while the first inputs stream in
                # (throwaway matmuls; results discarded).  Sized so the ramp
                # completes (~3us) and the last one ends right as the first
                # real chain's inputs land.
                for wi in range(2):
                    ps = ppsum.tile([P, 512], F32, tag="pp")
                    for _ in range(5):
                        nc.tensor.matmul(ps[:], junk_l[:], junk_r[:],
                                         start=True, stop=True)

                # k projection, own 1024-row half, kb-outer.  Chains evict
                # into a contiguous staging tile so each kb half goes to the
                # collective input as ONE contiguous 1MB DMA.  Staging rides
                # the scalar HWDGE queue (fast, and idle once wk is in);
                # the gpsimd queue only carries the AllGather doorbells.
                kstage = io.tile([P, KB, FO, 512], BF16)
                # kb0 runs as 256-wide chains: the first pass needs only the
                # first 256 kt columns (0.5MB) + the wk pair chunks, so real
                # work starts ~2us earlier while the rest streams in.
                for h in range(2):
                    for fo in range(FO):
                        ps = ppsum.tile([P, 512], F32, tag="pp")
                        for eo in range(EO):
                            nc.tensor.matmul(
                                ps[:, 0:256],
                                wk_sb[:, fo, eo, :],
                                kt_in[:, 0, eo, h * 256:(h + 1) * 256],
                                start=(eo == 0),
                                stop=(eo == EO - 1),
                            )
                        nc.scalar.activation(
                            kstage[:, 0, fo, h * 256:(h + 1) * 256],
                            ps[:, 0:256],
                            mybir.ActivationFunctionType.Identity,
                            bias=bk_sb[:, fo:fo + 1],
                        )
                nc.scalar.dma_start(out=cck_in[0][:], in_=kstage[:, 0])
                nc.gpsimd.collective_compute(
                    "AllGather",
                    mybir.AluOpType.bypass,
                    replica_groups=REPLICA_GROUPS,
                    ins=[cck_in[0][:]],
                    outs=[cck_out[0][:]],
                )
                for fo in range(FO):
                    ps = ppsum.tile([P, 512], F32, tag="pp")
                    for eo in range(EO):
                        nc.tensor.matmul(
                            ps[:],
                            wk_sb[:, fo, eo, :],
                            kt_in[:, 1, eo, :],
                            start=(eo == 0),
                            stop=(eo == EO - 1),
                        )
                    nc.scalar.activation(
                        kstage[:, 1, fo, :],
                        ps[:],
                        mybir.ActivationFunctionType.Identity,
                        bias=bk_sb[:, fo:fo + 1],
                    )
                nc.scalar.dma_start(out=cck_in[1][:], in_=kstage[:, 1])
                nc.gpsimd.collective_compute(
                    "AllGather",
                    mybir.AluOpType.bypass,
                    replica_groups=REPLICA_GROUPS,
                    ins=[cck_in[1][:]],
                    outs=[cck_out[1][:]],
                )

                # v projection, own half; evicts into v_proj[:, :KC//2, :].
                # Staged + exchanged in two 1MB halves so the first AllGather
                # launches ~14us earlier and each transfer stays small.
                for vh in range(2):
                    for kc in range(vh * (KC // 4), (vh + 1) * (KC // 4)):
                        for fb in range(2):
                            ps = ppsum.tile([P, 512], F32, tag="pp")
                            for eo in range(EO):
                                nc.tensor.matmul(
                                    ps[:],
                                    vt_all[:, kc, eo, :],
                                    wv_sb[:, eo, fb * 512:(fb + 1) * 512],
                                    start=(eo == 0),
                                    stop=(eo == EO - 1),
                                )
                            nc.vector.tensor_copy(
                                out=v_proj[:, kc, fb * 512:(fb + 1) * 512],
                                in_=ps[:],
                            )
                    nc.scalar.dma_start(
                        out=ccv_in[vh][:],
                        in_=v_proj[:, vh * (KC // 4):(vh + 1) * (KC // 4), :])
                    nc.gpsimd.collective_compute(
                        "AllGather",
                        mybir.AluOpType.bypass,
                        replica_groups=REPLICA_GROUPS,
                        ins=[ccv_in[vh][:]],
                        outs=[ccv_out[vh][:]],
                    )

                # q projection: psum[f128, q512] = sum_eo WqT[e,f].T @ qT[e,q]
                for fo in range(FO):
                    for qb in range(QB):
                        ps = ppsum.tile([P, 512], F32, tag="pp")
                        for eo in range(EO):
                            nc.tensor.matmul(
                                ps[:],
                                wq_sb[:, fo, eo, :],
                                qt_in[:, qb, eo, :],
                                start=(eo == 0),
                                stop=(eo == EO - 1),
                            )
                        nc.scalar.activation(
                            qt_proj[:, fo, qb * 512:(qb + 1) * 512],
                            ps[:],
                            mybir.ActivationFunctionType.Identity,
                            bias=bq_sb[:, fo:fo + 1],
                        )

                # Readbacks on the sync queue (behind the priming FIFO, which
                # has drained by the time each AllGather completes).  Local
                # halves too: rank order keeps the program SPMD-uniform.
                for kb in range(KB):
                    for r in range(2):
                        nc.sync.dma_start(
                            out=kt_proj[:, :,
                                        r * SQ + kb * 512:r * SQ + (kb + 1) * 512],
                            in_=cck_out[kb][r])
                if with_mask:
                    pass  # mask DMA issued in phase B (below), before v rb
                for vh in range(2):
                    for r in range(2):
                        nc.sync.dma_start(
                            out=v_proj[:, r * (KC // 2) + vh * (KC // 4):
                                       r * (KC // 2) + (vh + 1) * (KC // 4), :],
                            in_=ccv_out[vh][r])

            # ---------------- phase B: attention ----------------
            with (
                tc.tile_pool(name="phb", bufs=2) as phb,
                tc.tile_pool(name="outp", bufs=3) as outp,
                tc.tile_pool(name="rpool", bufs=8) as rpool,
            ):
                if with_mask:
                    mask_sb = phb.tile([P, KC, SQ], BF16, tag="mask", bufs=1)
                    nc.scalar.dma_start(out=mask_sb[:], in_=mask_d[:])

                # kc slots in collective-readback readiness order (the first
                # kb-half AllGather delivers slots {0-3, 8-11}).  With the
                # early staging this is normally all resident anyway; the
                # order just maximizes slack if a collective runs long.
                KC_ORDER = [0, 1, 2, 3, 8, 9, 10, 11, 4, 5, 6, 7, 12, 13, 14, 15]
                expTs, recips = [], []
                for qb in range(QB):
                    # scores + exp for this q 512-block
                    expT = phb.tile([P, KC, 512], BF16, tag="expT")
                    expTs.append(expT)
                    sums_acc = phb.tile([P, 512], F32, tag="sumacc")
                    for ki, kc in enumerate(KC_ORDER):
                        ps = ppsum.tile([P, 512], F32, tag="pp")
                        for fo in range(FO):
                            nc.tensor.matmul(
                                ps[:],
                                kt_proj[:, fo, kc * P:(kc + 1) * P],
                                qt_proj[:, fo, qb * 512:(qb + 1) * 512],
                                start=(fo == 0),
                                stop=(fo == FO - 1),
                            )
                        if with_mask:
                            nc.vector.tensor_scalar_mul(ps[:], ps[:], INV_SCALE)
                            nc.vector.tensor_add(
                                ps[:], ps[:],
                                mask_sb[:, kc, qb * 512:(qb + 1) * 512],
                            )
                            nc.scalar.activation(
                                expT[:, kc, :], ps[:],
                                mybir.ActivationFunctionType.Exp,
                            )
                        else:
                            nc.scalar.activation(
                                expT[:, kc, :], ps[:],
                                mybir.ActivationFunctionType.Exp,
                                scale=INV_SCALE,
                            )
                        # accumulate softmax denominators on DVE (frees the
                        # PE from the ones-column matmuls)
                        if ki == 0:
                            nc.vector.tensor_copy(
                                out=sums_acc[:], in_=expT[:, kc, :])
                        else:
                            nc.vector.tensor_add(
                                sums_acc[:], sums_acc[:], expT[:, kc, :])
                    # fold the partition axis with tiny fp32 ones-matmuls:
                    # psum[q128, 1] = sums_acc[:, qslice].T @ ones — lands
                    # directly in the per-partition layout the out-evict
                    # scale needs
                    qb_recips = []
                    for qi in range(4):
                        pf = spsum.tile([P, 1], F32, tag="pf")
                        nc.tensor.matmul(
                            pf[:],
                            sums_acc[:, qi * P:(qi + 1) * P],
                            ones_sb[:],
                            start=True, stop=True,
                        )
                        rt = rpool.tile([P, 1], F32, tag="recip")
                        nc.vector.reciprocal(rt[:], pf[:])
                        qb_recips.append(rt)
                    recips.append(qb_recips)

                # attn @ V per 128-row q tile.  pa chain (f 0:512) runs
                # fully, evicts, and stores its half while the pb chain
                # runs — halves the post-last-matmul tail.
                for qb in range(QB):
                    expT = expTs[qb]
                    for qi in range(4):
                        qg = qb * 4 + qi
                        rt = recips[qb][qi]
                        out_sb = outp.tile([P, E], BF16, tag="outsb")
                        pa = opsum_a.tile([P, 512], F32, tag="pa")
                        for ki, kc in enumerate(KC_ORDER):
                            nc.tensor.matmul(
                                pa[:], expT[:, kc, qi * P:(qi + 1) * P],
                                v_proj[:, kc, 0:512],
                                start=(ki == 0), stop=(ki == KC - 1))
                        nc.scalar.activation(
                            out_sb[:, 0:512], pa[:],
                            mybir.ActivationFunctionType.Copy,
                            scale=rt[:],
                        )
                        nc.sync.dma_start(
                            out=out_d[:, qg, 0:512], in_=out_sb[:, 0:512])
                        if qg == QB * 4 - 1:
                            # very last tile: the f 512:1024 half runs as two
                            # 256-wide chains so the first eviction + store
                            # overlap the second chain, shrinking the tail
                            for hb in range(2):
                                pool = opsum_a if hb == 0 else opsum_b
                                pbh = pool.tile([P, 512], F32,
                                                tag="pa" if hb == 0 else "pb")
                                sl = slice(512 + hb * 256, 768 + hb * 256)
                                for ki, kc in enumerate(KC_ORDER):
                                    nc.tensor.matmul(
                                        pbh[:, 0:256],
                                        expT[:, kc, qi * P:(qi + 1) * P],
                                        v_proj[:, kc, sl],
                                        start=(ki == 0), stop=(ki == KC - 1))
                                nc.scalar.activation(
                                    out_sb[:, sl], pbh[:, 0:256],
                                    mybir.ActivationFunctionType.Copy,
                                    scale=rt[:],
                                )
                                nc.sync.dma_start(
                                    out=out_d[:, qg, sl], in_=out_sb[:, sl])
                        else:
                            pb = opsum_b.tile([P, 512], F32, tag="pb")
                            for ki, kc in enumerate(KC_ORDER):
                                nc.tensor.matmul(
                                    pb[:], expT[:, kc, qi * P:(qi + 1) * P],
                                    v_proj[:, kc, 512:1024],
                                    start=(ki == 0), stop=(ki == KC - 1))
                            nc.scalar.activation(
                                out_sb[:, 512:1024], pb[:],
                                mybir.ActivationFunctionType.Copy,
                                scale=rt[:],
                            )
                            nc.sync.dma_start(
                                out=out_d[:, qg, 512:1024],
                                in_=out_sb[:, 512:1024])

    nc.compile()
    return nc


def _bf16_kb_tiled(x, nblk):
    """[R(e), C(s)] fp32 -> [128, nblk, R//128, C//nblk] bf16.

    partition = inner e index; axis1 = 512-col block of s; axis2 = e-outer.
    """
    r, c = x.shape
    w = c // nblk
    return (
        np.ascontiguousarray(x).astype(ml_dtypes.bfloat16)
        .reshape(r // P, P, nblk, w).transpose(1, 2, 0, 3).copy()
    )


def _prepare_in_maps(query, key, value, attn_mask, Wq, bq, Wk, bk, Wv, bv,
                     with_mask):
    query = np.asarray(query, np.float32)
    key = np.asarray(key, np.float32)
    value = np.asarray(value, np.float32)
    w_t = {}
    for name, w in (("wqt", Wq), ("wkt", Wk)):
        # W.T is [e(in), f(out)] -> [P(e-inner), FO, EO, 128(f-inner)]
        wT = np.ascontiguousarray(np.asarray(w, np.float32).T)
        w_t[name] = (
            wT.astype(ml_dtypes.bfloat16)
            .reshape(EO, P, FO, P).transpose(1, 2, 0, 3).copy()
        )
    wvT = np.ascontiguousarray(np.asarray(Wv, np.float32).T)
    w_t["wvt"] = (
        wvT.astype(ml_dtypes.bfloat16)
        .reshape(EO, P, E).transpose(1, 0, 2).copy()
    )
    bq_t = np.asarray(bq, np.float32).reshape(FO, P).T.copy()
    bk_t = np.asarray(bk, np.float32).reshape(FO, P).T.copy()

    in_maps = []
    for c in range(N_CORES):
        b, h = c // 2, c % 2
        qt = _bf16_kb_tiled(query[b, h * SQ:(h + 1) * SQ, :].T, QB)
        kt = _bf16_kb_tiled(key[b].T[:, h * SQ:(h + 1) * SQ], KB)
        # vt slabs for own k-half: [p(e-inner), kc_local, eo, w(k-inner)]
        vt = (
            np.ascontiguousarray(value[b].T[:, h * SQ:(h + 1) * SQ])
            .astype(ml_dtypes.bfloat16)
            .reshape(EO, P, KC // 2, P).transpose(1, 2, 0, 3).copy()
        )
        m = dict(qt=qt, kt=kt, vt=vt, bq=bq_t, bk=bk_t, **w_t)
        if with_mask:
            mt = np.asarray(attn_mask[b, h * SQ:(h + 1) * SQ, :], np.float32).T
            m["maskt"] = (
                mt.astype(ml_dtypes.bfloat16)
                .reshape(KC, P, SQ).transpose(1, 0, 2).copy()
            )
        in_maps.append(m)
    return in_maps


def _run(inputs, trace=False):
    with_mask = bool(np.any(np.asarray(inputs["attn_mask"])))
    key = with_mask
    if key not in _BUILD_CACHE:
        _BUILD_CACHE[key] = _build(with_mask)
    nc = _BUILD_CACHE[key]

    in_maps = _prepare_in_maps(with_mask=with_mask, **inputs)
    res = run_bass_kernel_spmd(nc, in_maps, core_ids=list(range(N_CORES)),
                               trace=trace)

    bv = np.asarray(inputs["bv"], np.float32)
    out = np.zeros((B, S, E), np.float32)
    for c in range(N_CORES):
        b, h = c // 2, c % 2
        oc = np.asarray(res.results[c]["out"], np.float32)  # [P, SQ//P, E]
        out[b, h * SQ:(h + 1) * SQ, :] = (
            oc.transpose(1, 0, 2).reshape(SQ, E) + bv[None, :]
        )
    return out, res


def kernel(**inputs) -> np.ndarray:
    out, _ = _run(inputs, trace=False)
    return out


# revision 31
# speedup vs baseline: 1.0096x; 1.0001x over previous
"""Self-contained Trainium2 Bass kernel for nn_AttentionModel (B=4, S=2048, E=1024).

Model: q/k/v linear projections + scaled-dot-product attention (scale = sqrt(E)).

Sharding (8 NeuronCores): core c handles batch b=c//2, query-row half h=c%2
(1024 q rows). k/v projections are split across the core pair (each projects
its own 1024 k-rows) and exchanged with a pair-local AllGather
(replica_groups [[0,1],[2,3],[4,5],[6,7]]); both halves are read back from
the collective output in absolute rank order, so the program stays fully
SPMD-uniform with no core-dependent addressing.

Device algorithm per core (all matmuls bf16 with fp32 PSUM accumulation):
  qT_proj[f,q]  = WqT.T @ qT_in   (+bq via ACT bias on eviction)
  kT_proj[f,k]  = WkT.T @ kT_in   (+bk)   [own half, then pair AllGather]
  v_proj [k,f]  = vT_in.T @ WvT   [own half, then pair AllGather;
                                   bias bv applied on host after gather]
  scoresT[k,q]  = kT_proj.T @ qT_proj          (per 128k x 512q psum tile)
  expT   [k,q]  = exp(scoresT / sqrt(E))       (ACT, no max-subtraction:
                                                logits are O(+-6), fp32-safe)
  out_un [q,f]  = expT.T @ v_proj              (accumulate over k chunks)
  sums   [q]    = DVE-accumulated exp tiles, partition-folded by a tiny
                  fp32 ones-matmul into per-partition [q,1] layout
  out    [q,f]  = out_un * (1/sums)            (per-partition ACT scale)

Scheduling notes (from perfetto analysis of the previous version):
  - Input priming is split across BOTH HWDGE engines (sync + scalar) so
    descriptor generation and transfers overlap; the k-proj inputs (kt
    halves on scalar, wk fo0-1 first on sync) land first so the PE starts
    ~13us in instead of ~21us.
  - k chains evict into a CONTIGUOUS staging tile (not kt_proj), so each
    kb half is staged to the collective input with ONE contiguous DMA and
    the AllGathers launch much earlier.  kt_proj is written only by the
    collective readbacks.  v staging is one contiguous DMA as well.
  - Readbacks ride the sync queue after priming drains; everything is
    resident long before the consuming phase, which removes the v-readback
    stall + half-rate region that used to sit at the attn@V entry.
  - attn@V runs the pa (f 0:512) chain fully, evicts + stores that half,
    then the pb chain, halving the post-last-matmul tail.

Host pre-tiles every input into fully-contiguous-per-partition bf16 SBUF
layouts, so the device performs no transposes or casts on the inputs.
"""

import sys

for _p in ("/opt/trn_rl_repo", "/root/.axon_site/_ro/trn_rl_repo"):
    if _p not in sys.path:
        sys.path.insert(0, _p)

import numpy as np
import ml_dtypes

import concourse.bacc as bacc
import concourse.mybir as mybir
import concourse.tile as tile
from concourse.bass_utils import run_bass_kernel_spmd

B, S, E = 4, 2048, 1024
P = 128
SQ = S // 2          # q rows per core
N_CORES = 8
EO = E // P          # 8  e-outer chunks
FO = E // P          # 8  f-outer chunks
KC = S // P          # 16 k-row chunks
QB = SQ // 512       # 2  q 512-blocks
KB = SQ // 512       # 2  k 512-blocks (own half)
INV_SCALE = float(1.0 / np.sqrt(np.float32(E)))

BF16 = mybir.dt.bfloat16
F32 = mybir.dt.float32

_BUILD_CACHE: dict = {}


def _build(with_mask: bool):
    nc = bacc.Bacc(
        "TRN2",
        target_bir_lowering=False,
        debug=False,
        enable_asserts=False,
        num_devices=N_CORES,
    )

    # Host-pretiled inputs; every transfer below is contiguous per partition.
    qt_d = nc.declare_dram_parameter("qt", [P, QB, EO, 512], BF16, isOutput=False)
    kt_d = nc.declare_dram_parameter("kt", [P, KB, EO, 512], BF16, isOutput=False)
    vt_d = nc.declare_dram_parameter("vt", [P, KC // 2, EO, P], BF16, isOutput=False)
    wqt_d = nc.declare_dram_parameter("wqt", [P, FO, EO, P], BF16, isOutput=False)
    wkt_d = nc.declare_dram_parameter("wkt", [P, FO, EO, P], BF16, isOutput=False)
    wvt_d = nc.declare_dram_parameter("wvt", [P, EO, E], BF16, isOutput=False)
    bq_d = nc.declare_dram_parameter("bq", [P, FO], F32, isOutput=False)
    bk_d = nc.declare_dram_parameter("bk", [P, FO], F32, isOutput=False)
    if with_mask:
        mask_d = nc.declare_dram_parameter("maskt", [P, KC, SQ], BF16, isOutput=False)
    # bf16 output: the attention output is an attn-weighted average (values
    # ~N(0, 0.03)); bf16 rounding adds ~0.4% relative error on top of the
    # ~0.6% bf16-compute error — far inside the 2e-2 gate — and halves the
    # output DMA traffic + final-store landing time.
    out_d = nc.declare_dram_parameter("out", [P, SQ // P, E], BF16, isOutput=True)

    # pair-AllGather staging buffers (internal DRAM)
    # NOTE: Shared-output collectives need >4-core groups; for 2-core pair
    # groups the output must stay a Local internal tensor. The k exchange is
    # split into two kb-halves so the first AllGather launches early and the
    # CC stream stays busy while phase A continues.
    cck_in = [nc.dram_tensor(f"cck_in{kb}", [P, FO, 512], BF16) for kb in range(KB)]
    cck_out = [nc.dram_tensor(f"cck_out{kb}", [2, P, FO, 512], BF16)
               for kb in range(KB)]
    ccv_in = [nc.dram_tensor(f"ccv_in{h}", [P, KC // 4, E], BF16) for h in range(2)]
    ccv_out = [nc.dram_tensor(f"ccv_out{h}", [2, P, KC // 4, E], BF16)
               for h in range(2)]
    REPLICA_GROUPS = [[0, 1], [2, 3], [4, 5], [6, 7]]

    with tile.TileContext(nc) as tc:
        with (
            tc.tile_pool(name="const", bufs=1) as const,
            tc.tile_pool(name="proj", bufs=1) as proj,
            tc.tile_pool(name="ppsum", bufs=3, space="PSUM") as ppsum,
            tc.tile_pool(name="opsum_a", bufs=2, space="PSUM") as opsum_a,
            tc.tile_pool(name="opsum_b", bufs=1, space="PSUM") as opsum_b,
            tc.tile_pool(name="spsum", bufs=2, space="PSUM") as spsum,
        ):
            # p-state warm-up fodder: the PE clock ramps for ~5us after an
            # idle stretch, so while the first inputs stream in we keep the
            # PE busy on throwaway matmuls and enter the first real chain at
            # full clock.  Memsets are PINNED to gpsimd and emitted first:
            # there they execute before the TileContext entry barrier
            # (~6.4us), letting the warm-up begin ~1us earlier.
            junk_l = const.tile([P, P], BF16)
            nc.gpsimd.memset(junk_l[:], 0.0)
            junk_r = const.tile([P, 512], BF16)
            nc.gpsimd.memset(junk_r[:], 0.0)
            ones_sb = const.tile([P, 1], F32)
            nc.gpsimd.memset(ones_sb[:], 1.0)
            # biases ride the (otherwise idle-at-start) gpsimd queue so the
            # HWDGE queues carry only the big input transfers
            bq_sb = const.tile([P, FO], F32)
            nc.gpsimd.dma_start(out=bq_sb[:], in_=bq_d[:])
            bk_sb = const.tile([P, FO], F32)
            nc.gpsimd.dma_start(out=bk_sb[:], in_=bk_d[:])

            # The k-proj gating inputs live in the outermost pool so their
            # DMAs issue before the io-pool-entry barrier (~1us earlier than
            # the rest of the priming).
            kt_in = const.tile([P, KB, EO, 512], BF16)
            wk_sb = const.tile([P, FO, EO, P], BF16)
            nc.sync.dma_start(out=kt_in[:, 0, :, 0:256],
                              in_=kt_d[:, 0, :, 0:256])
            nc.sync.dma_start(out=wk_sb[:, 0:1], in_=wkt_d[:, 0:1])
            nc.sync.dma_start(out=wk_sb[:, 1:2], in_=wkt_d[:, 1:2])
            for fp in range(1, 4):
                nc.sync.dma_start(
                    out=wk_sb[:, 2 * fp:2 * fp + 2],
                    in_=wkt_d[:, 2 * fp:2 * fp + 2])
            nc.sync.dma_start(out=kt_in[:, 0, :, 256:512],
                              in_=kt_d[:, 0, :, 256:512])
            nc.sync.dma_start(out=kt_in[:, 1], in_=kt_d[:, 1])

            # persistent projected tensors.  kt_proj / the gathered half of
            # v_proj are written ONLY by the collective readbacks.
            qt_proj = proj.tile([P, FO, SQ], BF16)   # [f-inner, fo, q]
            kt_proj = proj.tile([P, FO, S], BF16)    # [f-inner, fo, k]
            v_proj = proj.tile([P, KC, E], BF16)     # [k-inner, kc, f]

            # ---------------- phase A: projections ----------------
            with tc.tile_pool(name="io", bufs=1) as io:
                # ALL input priming on the sync HWDGE queue, in exact
                # consumption order — one queue at full HBM share, FIFO, so
                # each chunk lands just ahead of the chain that consumes it.
                # (Splitting across queues makes the queues COMPETE for HBM
                # bandwidth and the gating chunk lands later, which stalls
                # chains and drops the PE p-state — measured, not theory.)
                vt_all = io.tile([P, KC // 2, EO, P], BF16)
                nc.sync.dma_start(out=vt_all[:], in_=vt_d[:])
                wv_sb = io.tile([P, EO, E], BF16)
                nc.sync.dma_start(out=wv_sb[:], in_=wvt_d[:])
                wq_sb = io.tile([P, FO, EO, P], BF16)
                nc.sync.dma_start(out=wq_sb[:], in_=wqt_d[:])
                qt_in = io.tile([P, QB, EO, 512], BF16)
                nc.sync.dma_start(out=qt_in[:], in_=qt_d[:])

                # PE p-state warm-up while the first inputs stream in
                # (throwaway matmuls; results discarded).  Sized so the ramp
                # completes (~3us) and the last one ends right as the first
                # real chain's inputs land.
                for wi in range(2):
                    ps = ppsum.tile([P, 512], F32, tag="pp")
                    for _ in range(7 - wi):
                        nc.tensor.matmul(ps[:], junk_l[:], junk_r[:],
                                         start=True, stop=True)

                # k projection, own 1024-row half, kb-outer.  Chains evict
                # into a contiguous staging tile so each kb half goes to the
                # collective input as ONE contiguous 1MB DMA.  Staging rides
                # the scalar HWDGE queue (fast, and idle once wk is in);
                # the gpsimd queue only carries the AllGather doorbells.
                kstage = io.tile([P, KB, FO, 512], BF16)
                # kb0 runs as 256-wide chains: the first pass needs only the
                # first 256 kt columns (0.5MB) + the wk pair chunks, so real
                # work starts ~2us earlier while the rest streams in.
                for h in range(2):
                    for fo in range(FO):
                        ps = ppsum.tile([P, 512], F32, tag="pp")
                        for eo in range(EO):
                            nc.tensor.matmul(
                                ps[:, 0:256],
                                wk_sb[:, fo, eo, :],
                                kt_in[:, 0, eo, h * 256:(h + 1) * 256],
                                start=(eo == 0),
                                stop=(eo == EO - 1),
                            )
                        nc.scalar.activation(
                            kstage[:, 0, fo, h * 256:(h + 1) * 256],
                            ps[:, 0:256],
                            mybir.ActivationFunctionType.Identity,
                            bias=bk_sb[:, fo:fo + 1],
                        )
                nc.scalar.dma_start(out=cck_in[0][:], in_=kstage[:, 0])
                nc.gpsimd.collective_compute(
                    "AllGather",
                    mybir.AluOpType.bypass,
                    replica_groups=REPLICA_GROUPS,
                    ins=[cck_in[0][:]],
                    outs=[cck_out[0][:]],
                )
                for fo in range(FO):
                    ps = ppsum.tile([P, 512], F32, tag="pp")
                    for eo in range(EO):
                        nc.tensor.matmul(
                            ps[:],
                            wk_sb[:, fo, eo, :],
                            kt_in[:, 1, eo, :],
                            start=(eo == 0),
                            stop=(eo == EO - 1),
                        )
                    nc.scalar.activation(
                        kstage[:, 1, fo, :],
                        ps[:],
                        mybir.ActivationFunctionType.Identity,
                        bias=bk_sb[:, fo:fo + 1],
                    )
                nc.scalar.dma_start(out=cck_in[1][:], in_=kstage[:, 1])
                nc.gpsimd.collective_compute(
                    "AllGather",
                    mybir.AluOpType.bypass,
                    replica_groups=REPLICA_GROUPS,
                    ins=[cck_in[1][:]],
                    outs=[cck_out[1][:]],
                )

                # v projection, own half; evicts into v_proj[:, :KC//2, :].
                # Staged + exchanged in two 1MB halves so the first AllGather
                # launches ~14us earlier and each transfer stays small.
                for vh in range(2):
                    for kc in range(vh * (KC // 4), (vh + 1) * (KC // 4)):
                        for fb in range(2):
                            ps = ppsum.tile([P, 512], F32, tag="pp")
                            for eo in range(EO):
                                nc.tensor.matmul(
                                    ps[:],
                                    vt_all[:, kc, eo, :],
                                    wv_sb[:, eo, fb * 512:(fb + 1) * 512],
                                    start=(eo == 0),
                                    stop=(eo == EO - 1),
                                )
                            nc.vector.tensor_copy(
                                out=v_proj[:, kc, fb * 512:(fb + 1) * 512],
                                in_=ps[:],
                            )
                    nc.scalar.dma_start(
                        out=ccv_in[vh][:],
                        in_=v_proj[:, vh * (KC // 4):(vh + 1) * (KC // 4), :])
                    nc.gpsimd.collective_compute(
                        "AllGather",
                        mybir.AluOpType.bypass,
                        replica_groups=REPLICA_GROUPS,
                        ins=[ccv_in[vh][:]],
                        outs=[ccv_out[vh][:]],
                    )

                # q projection: psum[f128, q512] = sum_eo WqT[e,f].T @ qT[e,q]
                for fo in range(FO):
                    for qb in range(QB):
                        ps = ppsum.tile([P, 512], F32, tag="pp")
                        for eo in range(EO):
                            nc.tensor.matmul(
                                ps[:],
                                wq_sb[:, fo, eo, :],
                                qt_in[:, qb, eo, :],
                                start=(eo == 0),
                                stop=(eo == EO - 1),
                            )
                        nc.scalar.activation(
                            qt_proj[:, fo, qb * 512:(qb + 1) * 512],
                            ps[:],
                            mybir.ActivationFunctionType.Identity,
                            bias=bq_sb[:, fo:fo + 1],
                        )

                # Readbacks on the sync queue (behind the priming FIFO, which
                # has drained by the time each AllGather completes).  Local
                # halves too: rank order keeps the program SPMD-uniform.
                for kb in range(KB):
                    for r in range(2):
                        nc.sync.dma_start(
                            out=kt_proj[:, :,
                                        r * SQ + kb * 512:r * SQ + (kb + 1) * 512],
                            in_=cck_out[kb][r])
                if with_mask:
                    pass  # mask DMA issued in phase B (below), before v rb
                for vh in range(2):
                    for r in range(2):
                        nc.sync.dma_start(
                            out=v_proj[:, r * (KC // 2) + vh * (KC // 4):
                                       r * (KC // 2) + (vh + 1) * (KC // 4), :],
                            in_=ccv_out[vh][r])

            # ---------------- phase B: attention ----------------
            with (
                tc.tile_pool(name="phb", bufs=2) as phb,
                tc.tile_pool(name="outp", bufs=3) as outp,
                tc.tile_pool(name="rpool", bufs=8) as rpool,
            ):
                if with_mask:
                    mask_sb = phb.tile([P, KC, SQ], BF16, tag="mask", bufs=1)
                    nc.scalar.dma_start(out=mask_sb[:], in_=mask_d[:])

                # kc slots in collective-readback readiness order (the first
                # kb-half AllGather delivers slots {0-3, 8-11}).  With the
                # early staging this is normally all resident anyway; the
                # order just maximizes slack if a collective runs long.
                KC_ORDER = [0, 1, 2, 3, 8, 9, 10, 11, 4, 5, 6, 7, 12, 13, 14, 15]
                expTs, recips = [], []
                for qb in range(QB):
                    # scores + exp for this q 512-block
                    expT = phb.tile([P, KC, 512], BF16, tag="expT")
                    expTs.append(expT)
                    sums_acc = phb.tile([P, 512], F32, tag="sumacc")
                    for ki, kc in enumerate(KC_ORDER):
                        ps = ppsum.tile([P, 512], F32, tag="pp")
                        for fo in range(FO):
                            nc.tensor.matmul(
                                ps[:],
                                kt_proj[:, fo, kc * P:(kc + 1) * P],
                                qt_proj[:, fo, qb * 512:(qb + 1) * 512],
                                start=(fo == 0),
                                stop=(fo == FO - 1),
                            )
                        if with_mask:
                            nc.vector.tensor_scalar_mul(ps[:], ps[:], INV_SCALE)
                            nc.vector.tensor_add(
                                ps[:], ps[:],
                                mask_sb[:, kc, qb * 512:(qb + 1) * 512],
                            )
                            nc.scalar.activation(
                                expT[:, kc, :], ps[:],
                                mybir.ActivationFunctionType.Exp,
                            )
                        else:
                            nc.scalar.activation(
                                expT[:, kc, :], ps[:],
                                mybir.ActivationFunctionType.Exp,
                                scale=INV_SCALE,
                            )
                        # accumulate softmax denominators on DVE (frees the
                        # PE from the ones-column matmuls)
                        if ki == 0:
                            nc.vector.tensor_copy(
                                out=sums_acc[:], in_=expT[:, kc, :])
                        else:
                            nc.vector.tensor_add(
                                sums_acc[:], sums_acc[:], expT[:, kc, :])
                    # fold the partition axis with tiny fp32 ones-matmuls:
                    # psum[q128, 1] = sums_acc[:, qslice].T @ ones — lands
                    # directly in the per-partition layout the out-evict
                    # scale needs
                    qb_recips = []
                    for qi in range(4):
                        pf = spsum.tile([P, 1], F32, tag="pf")
                        nc.tensor.matmul(
                            pf[:],
                            sums_acc[:, qi * P:(qi + 1) * P],
                            ones_sb[:],
                            start=True, stop=True,
                        )
                        rt = rpool.tile([P, 1], F32, tag="recip")
                        nc.vector.reciprocal(rt[:], pf[:])
                        qb_recips.append(rt)
                    recips.append(qb_recips)

                # attn @ V per 128-row q tile.  pa chain (f 0:512) runs
                # fully, evicts, and stores its half while the pb chain
                # runs — halves the post-last-matmul tail.
                for qb in range(QB):
                    expT = expTs[qb]
                    for qi in range(4):
                        qg = qb * 4 + qi
                        rt = recips[qb][qi]
                        out_sb = outp.tile([P, E], BF16, tag="outsb")
                        pa = opsum_a.tile([P, 512], F32, tag="pa")
                        for ki, kc in enumerate(KC_ORDER):
                            nc.tensor.matmul(
                                pa[:], expT[:, kc, qi * P:(qi + 1) * P],
                                v_proj[:, kc, 0:512],
                                start=(ki == 0), stop=(ki == KC - 1))
                        nc.scalar.activation(
                            out_sb[:, 0:512], pa[:],
                            mybir.ActivationFunctionType.Copy,
                            scale=rt[:],
                        )
                        nc.sync.dma_start(
                            out=out_d[:, qg, 0:512], in_=out_sb[:, 0:512])
                        if qg == QB * 4 - 1:
                            # very last tile: the f 512:1024 half runs as two
                            # 256-wide chains so the first eviction + store
                            # overlap the second chain, shrinking the tail
                            for hb in range(2):
                                pool = opsum_a if hb == 0 else opsum_b
                                pbh = pool.tile([P, 512], F32,
                                                tag="pa" if hb == 0 else "pb")
                                sl = slice(512 + hb * 256, 768 + hb * 256)
                                for ki, kc in enumerate(KC_ORDER):
                                    nc.tensor.matmul(
                                        pbh[:, 0:256],
                                        expT[:, kc, qi * P:(qi + 1) * P],
                                        v_proj[:, kc, sl],
                                        start=(ki == 0), stop=(ki == KC - 1))
                                nc.scalar.activation(
                                    out_sb[:, sl], pbh[:, 0:256],
                                    mybir.ActivationFunctionType.Copy,
                                    scale=rt[:],
                                )
                                nc.sync.dma_start(
                                    out=out_d[:, qg, sl], in_=out_sb[:, sl])
                        else:
                            pb = opsum_b.tile([P, 512], F32, tag="pb")
                            for ki, kc in enumerate(KC_ORDER):
                                nc.tensor.matmul(
                                    pb[:], expT[:, kc, qi * P:(qi + 1) * P],
                                    v_proj[:, kc, 512:1024],
                                    start=(ki == 0), stop=(ki == KC - 1))
                            nc.scalar.activation(
                                out_sb[:, 512:1024], pb[:],
                                mybir.ActivationFunctionType.Copy,
                                scale=rt[:],
                            )
                            nc.sync.dma_start(
                                out=out_d[:, qg, 512:1024],
                                in_=out_sb[:, 512:1024])

    nc.compile()
    return nc


def _bf16_kb_tiled(x, nblk):
    """[R(e), C(s)] fp32 -> [128, nblk, R//128, C//nblk] bf16.

    partition = inner e index; axis1 = 512-col block of s; axis2 = e-outer.
    """
    r, c = x.shape
    w = c // nblk
    return (
        np.ascontiguousarray(x).astype(ml_dtypes.bfloat16)
        .reshape(r // P, P, nblk, w).transpose(1, 2, 0, 3).copy()
    )


def _prepare_in_maps(query, key, value, attn_mask, Wq, bq, Wk, bk, Wv, bv,
                     with_mask):
    query = np.asarray(query, np.float32)
    key = np.asarray(key, np.float32)
    value = np.asarray(value, np.float32)
    w_t = {}
    for name, w in (("wqt", Wq), ("wkt", Wk)):
        # W.T is [e(in), f(out)] -> [P(e-inner), FO, EO, 128(f-inner)]
        wT = np.ascontiguousarray(np.asarray(w, np.float32).T)
        w_t[name] = (
            wT.astype(ml_dtypes.bfloat16)
            .reshape(EO, P, FO, P).transpose(1, 2, 0, 3).copy()
        )
    wvT = np.ascontiguousarray(np.asarray(Wv, np.float32).T)
    w_t["wvt"] = (
        wvT.astype(ml_dtypes.bfloat16)
        .reshape(EO, P, E).transpose(1, 0, 2).copy()
    )
    bq_t = np.asarray(bq, np.float32).reshape(FO, P).T.copy()
    bk_t = np.asarray(bk, np.float32).reshape(FO, P).T.copy()

    in_maps = []
    for c in range(N_CORES):
        b, h = c // 2, c % 2
        qt = _bf16_kb_tiled(query[b, h * SQ:(h + 1) * SQ, :].T, QB)
        kt = _bf16_kb_tiled(key[b].T[:, h * SQ:(h + 1) * SQ], KB)
        # vt slabs for own k-half: [p(e-inner), kc_local, eo, w(k-inner)]
        vt = (
            np.ascontiguousarray(value[b].T[:, h * SQ:(h + 1) * SQ])
            .astype(ml_dtypes.bfloat16)
            .reshape(EO, P, KC // 2, P).transpose(1, 2, 0, 3).copy()
        )
        m = dict(qt=qt, kt=kt, vt=vt, bq=bq_t, bk=bk_t, **w_t)
        if with_mask:
            mt = np.asarray(attn_mask[b, h * SQ:(h + 1) * SQ, :], np.float32).T
            m["maskt"] = (
                mt.astype(ml_dtypes.bfloat16)
                .reshape(KC, P, SQ).transpose(1, 0, 2).copy()
            )
        in_maps.append(m)
    return in_maps


def _run(inputs, trace=False):
    with_mask = bool(np.any(np.asarray(inputs["attn_mask"])))
    key = with_mask
    if key not in _BUILD_CACHE:
        _BUILD_CACHE[key] = _build(with_mask)
    nc = _BUILD_CACHE[key]

    in_maps = _prepare_in_maps(with_mask=with_mask, **inputs)
    res = run_bass_kernel_spmd(nc, in_maps, core_ids=list(range(N_CORES)),
                               trace=trace)

    bv = np.asarray(inputs["bv"], np.float32)
    out = np.zeros((B, S, E), np.float32)
    for c in range(N_CORES):
        b, h = c // 2, c % 2
        oc = np.asarray(res.results[c]["out"], np.float32)  # [P, SQ//P, E]
        out[b, h * SQ:(h + 1) * SQ, :] = (
            oc.transpose(1, 0, 2).reshape(SQ, E) + bv[None, :]
        )
    return out, res


def kernel(**inputs) -> np.ndarray:
    out, _ = _run(inputs, trace=False)
    return out
